# revision 1
# baseline (speedup 1.0000x reference)
"""Bass/Trainium2 kernel for a 2-layer GCN encoder (nn_GCNEncoder).

Computation (per reference):
  h = relu(LN(segment_sum(ew * (x@W1)[src], dst) + b1; g1, be1))
  h = relu(LN(segment_sum(ew * (h@W2)[src], dst) + b2; g2, be2))

Strategy (8 NeuronCores, node sharding by dst):
  - Each core owns a contiguous shard of 6250 dst nodes, processing the
    ~100k edges that target its shard.
  - Gather x[src] rows with GPSIMD dma_gather (bf16, 256B rows) into SBUF
    chunk tiles of 128 edges.
  - Scatter-add via TensorE "segment matmul": for each chunk, an S matrix
    with S[e, dstl] = ew[e] (built on DVE with one dual-op tensor_scalar
    against an iota constant) gives B_T += X_chunk.T @ S accumulated in
    PSUM per 128-dst block.  agg = B_T.T @ [W | mean(W)] + [b | mean(b)]
    (second K=1 matmul adds the bias row), which also yields the LayerNorm
    mean for free in column 128.
  - LayerNorm + ReLU on DVE/ACT per block; layer-1 output is written
    bf16 to a DRAM bounce and AllGather'd (the collective is the only
    cross-core sync needed), layer 2 gathers from the gathered buffer.
  - Edges are sorted by dst block and split lo/hi on the gather row index
    because dma_gather indices are int16 (<32768).  Chunk counts per
    (block, half) are padded to the max over cores so all 8 cores share
    one SPMD program.
"""

import os
import numpy as np
import ml_dtypes

import concourse.bass as bass
import concourse.bacc as bacc
import concourse.mybir as mybir
import concourse.tile as tile
from concourse.bass_utils import run_bass_kernel_spmd

BF16 = mybir.dt.bfloat16
F32 = mybir.dt.float32
I16 = mybir.dt.int16

N_NODES = 50000
D = 128
P = 8
HALF = 32768
LN_EPS = 1e-5
PIECE = 32  # chunks per dma_gather call
ABLATE = set()  # cost-model ablation flags (timing experiments only)


def _cdiv(a, b):
    return (a + b - 1) // b


def _subsplit(nblk, nsub=1):
    """Split nblk blocks into <=nsub contiguous groups (for sub-AllGathers).
    Returns list of (j0, nb) and row bases into the cc table."""
    nsub = min(nsub, nblk)
    sizes = [len(a) for a in np.array_split(np.arange(nblk), nsub)]
    out = []
    j0 = 0
    for nb in sizes:
        out.append((j0, nb))
        j0 += nb
    return out


class Schedule:
    """Per-layer SPMD gather/matmul schedule shared by all cores."""

    def __init__(self, nlo, nhi):
        self.nlo = nlo            # [NBLK] lo chunks per block
        self.nhi = nhi            # [NBLK] hi chunks per block
        self.NLO = int(nlo.sum())
        self.NHI = int(nhi.sum())


def _make_schedule_and_arrays(rows_idx, dstl, ew, blk, nblk, half_thresh, ncores,
                              core_of_edge):
    """rows_idx: gather-table row index per edge (global). Returns
    (Schedule, per-core dict of arrays)."""
    half = (rows_idx >= half_thresh).astype(np.int64)
    # counts[core, blk, half]
    key = (core_of_edge * nblk + blk) * 2 + half
    counts = np.bincount(key, minlength=ncores * nblk * 2).reshape(ncores, nblk, 2)
    mx = counts.max(axis=0)  # [nblk, 2]
    nlo = _cdiv(mx[:, 0], 128)
    nhi = _cdiv(mx[:, 1], 128)
    # every block needs at least one chunk so PSUM gets cleared
    force = (nlo + nhi) == 0
    nlo[force] = 1
    sched = Schedule(nlo, nhi)

    lo_base = np.concatenate([[0], np.cumsum(nlo)])[:-1] * 128
    hi_base = np.concatenate([[0], np.cumsum(nhi)])[:-1] * 128

    per_core = []
    for c in range(ncores):
        m = core_of_edge == c
        r, dl, w, b, h = rows_idx[m], dstl[m], ew[m], blk[m], half[m]
        # adjust hi rows to offset table
        r = np.where(h == 1, r - half_thresh, r)
        order = np.lexsort((r, h, b))
        r, dl, w, b, h = r[order], dl[order], w[order], b[order], h[order]
        # position within (blk, half) group
        gkey = b * 2 + h
        gcnt = np.bincount(gkey, minlength=nblk * 2)
        gstart = np.concatenate([[0], np.cumsum(gcnt)])[:-1]
        pos_in_group = np.arange(len(r)) - gstart[gkey]
        base = np.where(h == 0, lo_base[b], hi_base[b])
        pos = base + pos_in_group

        idx_lo = np.zeros(sched.NLO * 128, np.int16)
        dstl_lo = np.zeros(sched.NLO * 128, np.float32)
        ew_lo = np.zeros(sched.NLO * 128, np.float32)
        idx_hi = np.zeros(max(sched.NHI, 1) * 128, np.int16)
        dstl_hi = np.zeros(max(sched.NHI, 1) * 128, np.float32)
        ew_hi = np.zeros(max(sched.NHI, 1) * 128, np.float32)
        lo = h == 0
        idx_lo[pos[lo]] = r[lo]
        dstl_lo[pos[lo]] = dl[lo]
        ew_lo[pos[lo]] = w[lo]
        hi = ~lo
        idx_hi[pos[hi]] = r[hi]
        dstl_hi[pos[hi]] = dl[hi]
        ew_hi[pos[hi]] = w[hi]
        per_core.append(dict(idx_lo=idx_lo, dstl_lo=dstl_lo, ew_lo=ew_lo,
                             idx_hi=idx_hi, dstl_hi=dstl_hi, ew_hi=ew_hi))
    return sched, per_core


def _pack_idxs(idx):
    """int16 (n,) with n%16==0 -> [128, n//16] wrapped in 16 partitions,
    replicated 8x (one copy per Q7 core)."""
    n = len(idx)
    a = np.asarray(idx, np.int16).reshape(n // 16, 16).T
    return np.ascontiguousarray(np.tile(a, (8, 1)))


def _pack_cols(v):
    """(nch*128,) -> [128, nch]: column j = values of chunk j."""
    return np.ascontiguousarray(np.asarray(v).reshape(-1, 128).T)


def build_kernel(n_nodes, nshard, ncores, sched1, sched2, piece=PIECE,
                 half_thresh=HALF, dma_scratch=16384, n_queues=1, sim1=False,
                 ring_bufs=2, mask_bufs=8, work_bufs=4, bt_bufs=4, agg_bufs=2):
    nblk = _cdiv(nshard, 128)
    npad = nblk * 128
    nc = bacc.Bacc("TRN2", target_bir_lowering=False, debug=False,
                   num_devices=1 if sim1 else ncores,
                   dynamic_dma_scratch_size=dma_scratch,
                   num_swdge_queues=n_queues)

    def din(name, shape, dt):
        return nc.dram_tensor(name, shape, dt, kind="ExternalInput")

    x = din("xb", [n_nodes, D], BF16)
    iota = din("iota", [128, 128], BF16)
    ones1 = din("ones1", [1, 128], BF16)
    w_aug = [din("w1a", [D, D + 1], BF16), din("w2a", [D, D + 1], BF16)]
    b_aug = [din("b1a", [1, D + 1], BF16), din("b2a", [1, D + 1], BF16)]
    g_rep = [din("g1r", [128, D], F32), din("g2r", [128, D], F32)]
    be_rep = [din("be1r", [128, D], F32), din("be2r", [128, D], F32)]

    idx_t = {}
    dstl_t = {}
    ew_t = {}
    scheds = {1: sched1, 2: sched2}
    for l in (1, 2):
        s = scheds[l]
        for st, nch in (("lo", s.NLO), ("hi", s.NHI)):
            nch_a = max(nch, 1)
            idx_t[l, st] = din(f"idx{l}{st}", [128, nch_a * 8], I16)
            dstl_t[l, st] = din(f"dstl{l}{st}", [128, nch_a], F32)
            ew_t[l, st] = din(f"ew{l}{st}", [128, nch_a], F32)

    out = nc.dram_tensor("out", [nshard, D], F32, kind="ExternalOutput")
    chain_in = nc.dram_tensor("chain_in", [1, 128], F32, kind="ExternalInput")
    chain_out = nc.dram_tensor("chain_out", [1, 128], F32, kind="ExternalOutput")
    subs = _subsplit(nblk)
    cc_ins = [nc.dram_tensor(f"cc_in{s}", [128, nb * 128], BF16, kind="Internal")
              for s, (j0, nb) in enumerate(subs)]
    cc_out = nc.dram_tensor("cc_out", [128 * ncores * npad // D, D], BF16,
                            kind="Internal", addr_space="Shared")
    cc_view = cc_out.ap()

    with tile.TileContext(nc) as tc:
        with (
            tc.tile_pool(name="const", bufs=1) as const,
            tc.tile_pool(name="ring", bufs=ring_bufs) as ring,
            tc.tile_pool(name="mask", bufs=mask_bufs) as maskp,
            tc.tile_pool(name="work", bufs=work_bufs) as work,
            tc.tile_pool(name="stat", bufs=16) as stat,
            tc.tile_pool(name="btps", bufs=bt_bufs, space="PSUM") as btps,
            tc.tile_pool(name="aggps", bufs=agg_bufs, space="PSUM") as aggps,
        ):
            iota_t = const.tile_from(iota.ap(), name="iota_t")
            zero_t = const.tile([128, 1], F32, tag="zero_t")
            nc.vector.memset(zero_t[:], 0.0)
            eps_t = const.tile([128, 1], F32, tag="eps_t")
            nc.vector.memset(eps_t[:], LN_EPS)
            ones1_t = const.tile_from(ones1.ap(), name="ones1_t")
            w_t = {l: const.tile_from(w_aug[l - 1].ap(), name=f"w{l}_t")
                   for l in (1, 2)}
            b_t = {l: const.tile_from(b_aug[l - 1].ap(), name=f"b{l}_t")
                   for l in (1, 2)}
            g_t = {l: const.tile_from(g_rep[l - 1].ap(), name=f"g{l}_t")
                   for l in (1, 2)}
            be_t = {l: const.tile_from(be_rep[l - 1].ap(), name=f"be{l}_t")
                    for l in (1, 2)}
            idx_s = {k: const.tile_from(v.ap(), name=f"idx_s{k[0]}{k[1]}")
                     for k, v in idx_t.items()}
            dstl_s = {k: const.tile_from(v.ap(), name=f"dstl_s{k[0]}{k[1]}")
                      for k, v in dstl_t.items()}
            ew_s = {k: const.tile_from(v.ap(), name=f"ew_s{k[0]}{k[1]}")
                    for k, v in ew_t.items()}

            h2_all = const.tile([128, nblk, 128], BF16, tag="h2all")
            out_all = const.tile([128, nblk, 128], F32, tag="outall")

            def emit_gathers(l, st, nch, src_ap):
                """One dma_gather per piece; returns list of piece tiles."""
                pieces = []
                for p0 in range(0, nch, piece):
                    k = min(piece, nch - p0)
                    t = ring.tile([128, piece, 128], BF16, tag=f"ring{st}")
                    if "nogather" in ABLATE:
                        nc.sync.dma_start(t[:, :k, :],
                                          src_ap[0:k * 128, :].rearrange(
                                              "(a p) f -> p a f", p=128))
                    else:
                        nc.gpsimd.dma_gather(
                            t[:, :k, :], src_ap,
                            idx_s[l, st][:, p0 * 8:(p0 + k) * 8],
                            k * 128, k * 128, D,
                            single_packet=(os.environ.get("SP", "0") == "1"),
                            queue_num=(len(pieces) % nc.num_swdge_queues))
                    pieces.append(t)
                return pieces

            def do_layer(l, lo_src, hi_src, write_out):
                s = scheds[l]
                pieces = {"lo": emit_gathers(l, "lo", s.NLO, lo_src)}
                if s.NHI:
                    pieces["hi"] = emit_gathers(l, "hi", s.NHI, hi_src)
                lo_pos = hi_pos = 0
                for b in range(nblk):
                    jobs = [("lo", lo_pos + i) for i in range(s.nlo[b])] + \
                           [("hi", hi_pos + i) for i in range(s.nhi[b])]
                    lo_pos += s.nlo[b]
                    hi_pos += s.nhi[b]
                    bt = btps.tile([128, 128], F32, tag="bt")
                    for ji, (st, pos) in enumerate(jobs):
                        if "nomask" in ABLATE:
                            S = iota_t
                        else:
                            S = maskp.tile([128, 128], BF16, tag="S")
                            nc.vector.tensor_scalar(
                                S[:], iota_t[:],
                                dstl_s[l, st][:, pos:pos + 1],
                                ew_s[l, st][:, pos:pos + 1],
                                mybir.AluOpType.is_equal, mybir.AluOpType.mult)
                        src_tile = pieces[st][pos // piece]
                        if "nomm" not in ABLATE:
                            nc.tensor.matmul(
                                bt[:], src_tile[:, pos % piece, :], S[:],
                                start=(ji == 0), stop=(ji == len(jobs) - 1))
                        elif ji == 0:
                            nc.tensor.matmul(
                                bt[:], src_tile[:, pos % piece, :], S[:],
                                start=True, stop=True)
                    btt = work.tile([128, 128], BF16, tag="btt")
                    nc.scalar.copy(btt[:], bt[:])
                    agg = aggps.tile([128, D + 1], F32, tag="agg")
                    nc.tensor.matmul(agg[:], btt[:], w_t[l][:],
                                     start=True, stop=False)
                    nc.tensor.matmul(agg[:], ones1_t[:], b_t[l][:],
                                     start=False, stop=True)
                    # LayerNorm + relu
                    if "noln" in ABLATE:
                        hbq = work.tile([128, 128], F32, tag="hb")
                        nc.vector.tensor_copy(hbq[:], agg[:, 0:D])
                        write_out(b, hbq)
                        continue
                    mu = stat.tile([128, 1], F32, tag="mu")
                    nc.scalar.copy(mu[:], agg[:, D:D + 1])
                    t_ = work.tile([128, 128], F32, tag="t")
                    nc.vector.tensor_scalar(t_[:], agg[:, 0:D], mu[:], None,
                                            mybir.AluOpType.subtract)
                    sq = work.tile([128, 128], F32, tag="sq")
                    var = stat.tile([128, 1], F32, tag="var")
                    nc.scalar.activation(sq[:], t_[:],
                                         mybir.ActivationFunctionType.Square,
                                         bias=zero_t[:], accum_out=var[:])
                    sd = stat.tile([128, 1], F32, tag="sd")
                    nc.scalar.activation(sd[:], var[:],
                                         mybir.ActivationFunctionType.Sqrt,
                                         bias=eps_t[:], scale=1.0 / D)
                    rstd = stat.tile([128, 1], F32, tag="rstd")
                    nc.vector.reciprocal(rstd[:], sd[:])
                    hn = work.tile([128, 128], F32, tag="hn")
                    nc.vector.tensor_scalar(hn[:], t_[:], rstd[:], None,
                                            mybir.AluOpType.mult)
                    hg = work.tile([128, 128], F32, tag="hg")
                    nc.vector.tensor_tensor(hg[:], hn[:], g_t[l][:],
                                            mybir.AluOpType.mult)
                    hb = work.tile([128, 128], F32, tag="hb")
                    nc.vector.tensor_tensor(hb[:], hg[:], be_t[l][:],
                                            mybir.AluOpType.add)
                    write_out(b, hb)

            sub_end = {j0 + nb - 1: s for s, (j0, nb) in enumerate(subs)}

            def l1_write(b, hb):
                nc.scalar.activation(h2_all[:, b, :], hb[:],
                                     mybir.ActivationFunctionType.Relu,
                                     bias=zero_t[:])
                if b in sub_end:
                    s = sub_end[b]
                    j0, nb = subs[s]
                    nc.sync.dma_start(cc_ins[s].ap(), h2_all[:, j0:j0 + nb, :])
                    base = sum(128 * ncores * subs[t][1] for t in range(s))
                    out_view = cc_out.ap()[base:base + 128 * ncores * nb, :] \
                        .rearrange("(q j) f -> q (j f)", j=nb)
                    if sim1:
                        nc.sync.dma_start(out_view[0:128, :], cc_ins[s].ap())
                    else:
                        nc.gpsimd.collective_compute(
                            "AllGather", mybir.AluOpType.bypass,
                            replica_groups=[list(range(ncores))],
                            ins=[cc_ins[s].ap()], outs=[out_view])

            def l2_write(b, hb):
                nc.scalar.activation(out_all[:, b, :], hb[:],
                                     mybir.ActivationFunctionType.Relu,
                                     bias=zero_t[:])

            do_layer(1, x.ap(), x.ap()[half_thresh:n_nodes, :], l1_write)

            nrows_cc = 128 * ncores * nblk
            do_layer(2, cc_view[0:min(half_thresh, nrows_cc), :],
                     cc_view[half_thresh:nrows_cc, :] if nrows_cc > half_thresh
                     else cc_view[0:nrows_cc, :],
                     l2_write)

            # write the output shard (node-major [nshard, D])
            nfull = (nshard // 128) * 128
            if nfull:
                nc.sync.dma_start(
                    out.ap()[0:nfull, :].rearrange("(j p) f -> p j f", p=128),
                    out_all[:, 0:nshard // 128, :])
            rem = nshard - nfull
            if rem:
                nc.sync.dma_start(out.ap()[nfull:nshard, :],
                                  out_all[0:rem, nshard // 128, :])
            cht = const.tile([1, 128], F32, tag="cht")
            nc.sync.dma_start(cht[:], chain_in.ap())
            nc.sync.dma_start(chain_out.ap(), cht[:])
    nc.compile()
    return nc


def _phi(n, nshard, nblk, ncores=P):
    """gather-table row in cc_out for global node n (sub-major layout)."""
    subs = _subsplit(nblk)
    r = n // nshard
    ld = n % nshard
    j = ld // 128
    p = ld % 128
    j0s = np.array([s[0] for s in subs])
    nbs = np.array([s[1] for s in subs])
    bases = np.concatenate([[0], np.cumsum(nbs * 128 * ncores)])[:-1]
    si = np.searchsorted(j0s, j, side="right") - 1
    return bases[si] + (r * 128 + p) * nbs[si] + (j - j0s[si])


def prepare(x, edge_index, edge_weight, W1, b1, W2, b2, g1, be1, g2, be2,
            n_nodes=N_NODES, ncores=P, half_thresh=HALF):
    """Host-side prep: returns (sched1, sched2, in_maps)."""
    nshard = n_nodes // ncores
    nblk = _cdiv(nshard, 128)

    src = np.asarray(edge_index[0]).astype(np.int64)
    dst = np.asarray(edge_index[1]).astype(np.int64)
    ew = np.asarray(edge_weight, np.float32)
    core_of_edge = dst // nshard
    ld = dst % nshard
    blk = ld // 128
    dstl = (ld % 128).astype(np.float32)

    sched1, pc1 = _make_schedule_and_arrays(
        src, dstl, ew, blk, nblk, half_thresh, ncores, core_of_edge)
    phi = _phi(src, nshard, nblk)
    sched2, pc2 = _make_schedule_and_arrays(
        phi, dstl, ew, blk, nblk, half_thresh, ncores, core_of_edge)

    xb = np.asarray(x, np.float32).astype(ml_dtypes.bfloat16)
    iota = np.tile(np.arange(128, dtype=np.float32), (128, 1)).astype(ml_dtypes.bfloat16)
    ones1 = np.ones((1, 128), ml_dtypes.bfloat16)

    def aug_w(W):
        W = np.asarray(W, np.float32)
        return np.concatenate([W, W.mean(axis=1, keepdims=True)], 1).astype(ml_dtypes.bfloat16)

    def aug_b(b):
        b = np.asarray(b, np.float32)
        return np.concatenate([b, [b.mean()]])[None, :].astype(ml_dtypes.bfloat16)

    base = {
        "xb": xb, "iota": iota, "ones1": ones1,
        "chain_in": np.zeros((1, 128), np.float32),
        "w1a": aug_w(W1), "w2a": aug_w(W2),
        "b1a": aug_b(b1), "b2a": aug_b(b2),
        "g1r": np.tile(np.asarray(g1, np.float32), (128, 1)),
        "g2r": np.tile(np.asarray(g2, np.float32), (128, 1)),
        "be1r": np.tile(np.asarray(be1, np.float32), (128, 1)),
        "be2r": np.tile(np.asarray(be2, np.float32), (128, 1)),
    }
    in_maps = []
    for c in range(ncores):
        m = dict(base)
        for l, pc in ((1, pc1), (2, pc2)):
            d = pc[c]
            m[f"idx{l}lo"] = _pack_idxs(d["idx_lo"])
            m[f"dstl{l}lo"] = _pack_cols(d["dstl_lo"])
            m[f"ew{l}lo"] = _pack_cols(d["ew_lo"])
            m[f"idx{l}hi"] = _pack_idxs(d["idx_hi"])
            m[f"dstl{l}hi"] = _pack_cols(d["dstl_hi"])
            m[f"ew{l}hi"] = _pack_cols(d["ew_hi"])
        in_maps.append(m)
    return sched1, sched2, in_maps


_CACHE = {}


def kernel(x, edge_index, edge_weight, W1, b1, W2, b2, g1, be1, g2, be2):
    n_nodes = int(np.asarray(x).shape[0])
    nshard = n_nodes // P
    sched1, sched2, in_maps = prepare(
        x, edge_index, edge_weight, W1, b1, W2, b2, g1, be1, g2, be2,
        n_nodes=n_nodes)
    key = (n_nodes, tuple(sched1.nlo), tuple(sched1.nhi),
           tuple(sched2.nlo), tuple(sched2.nhi))
    if key not in _CACHE:
        _CACHE[key] = build_kernel(n_nodes, nshard, P, sched1, sched2,
                                   ring_bufs=3)
    nc = _CACHE[key]
    res = run_bass_kernel_spmd(nc, in_maps, core_ids=list(range(P)))
    return np.concatenate([res.results[c]["out"] for c in range(P)], axis=0)


if __name__ == "__main__":
    import reference
    inputs = {k: np.asarray(v) for k, v in reference.setup_inputs().items()}
    out = kernel(**inputs)
    exp = np.asarray(reference.reference(**inputs))
    err = np.abs(out - exp).max()
    denom = np.abs(exp).max()
    print("max abs err:", err, "rel:", err / denom)



# revision 7
# speedup vs baseline: 6.7335x; 6.7335x over previous
"""Bass/Trainium2 kernel for a 2-layer GCN encoder (nn_GCNEncoder).

Computation (per reference):
  h = relu(LN(segment_sum(ew * (x@W1)[src], dst) + b1; g1, be1))
  h = relu(LN(segment_sum(ew * (h@W2)[src], dst) + b2; g2, be2))

Design notes (v2 — staging-optimized):
  The execution channel charges ~1 ms per input tensor per exec plus byte
  cost for multi-MB tensors, so the kernel stages exactly ONE packed int16
  blob (~2.6 MB/core) plus the tiny chain tensor, and returns bf16 output
  (host casts to f32).

  - x is sharded by node rows (1.6 MB/core bf16) and AllGather'd on device
    into a shared DRAM buffer laid out in "cc" order (row phi(n) per node n).
    Layer 1 then aggregates RAW x rows per dst block (W1 applied after
    aggregation — linearity), which makes layer 1 structurally identical to
    layer 2 and lets ONE edge schedule (idx/dstl/ew tables) serve both
    layers.
  - Per dst block of 128 nodes: gather 128-edge chunks of source rows with
    GPSIMD dma_gather (bf16, 256B rows), build S[e, dstl] = ew[e] with one
    dual-op DVE tensor_scalar against an iota constant, accumulate
    B_T += X_chunk.T @ S in PSUM, then agg = B_T.T @ [W | mean(W)] + b_aug
    (the K=1 matmul adds bias and yields the LayerNorm mean for free).
  - dma_gather descriptor generation runs on Q7 core pair (2q, 2q+1) for
    queue q; gather pieces are spread round-robin over 4 SWDGE queues, and
    the idx table stages only the 2 copies the assigned queue's cores read
    (packed at partitions [32q, 32q+32)) instead of 8.
  - Edges are sorted by dst block and split lo/hi on the gather row index
    (dma_gather indices are int16 (<32768)).  Chunk counts per (block,
    half) are padded to the max over cores so all 8 cores share one SPMD
    program.
"""

import numpy as np
import ml_dtypes

import concourse.bass as bass
import concourse.bacc as bacc
import concourse.mybir as mybir
import concourse.tile as tile
from concourse.bass_utils import run_bass_kernel_spmd

BF16 = mybir.dt.bfloat16
F32 = mybir.dt.float32
I16 = mybir.dt.int16

N_NODES = 50000
D = 128
P = 8
HALF = 32768
LN_EPS = 1e-5
PIECE = 32   # chunks per dma_gather call
NQ = 4       # SWDGE queues (Q7 core pairs) used round-robin


def _cdiv(a, b):
    return (a + b - 1) // b


class Schedule:
    """SPMD gather/matmul schedule shared by all cores and both layers."""

    def __init__(self, nlo, nhi, piece=PIECE, nq=NQ):
        self.nlo = nlo            # [NBLK] lo chunks per block
        self.nhi = nhi            # [NBLK] hi chunks per block
        self.NLO = int(nlo.sum())
        self.NHI = int(nhi.sum())
        self.NCH = self.NLO + self.NHI
        self.piece = piece
        # pieces: (st, p0, k, queue, qcol); qcol = column offset of this
        # piece's indices inside its queue's idx stream.
        self.pieces = []
        raw = []
        for st, n in (("lo", self.NLO), ("hi", self.NHI)):
            for p0 in range(0, n, piece):
                raw.append((st, p0, min(piece, n - p0)))
        qcols = [0] * nq
        for i, (st, p0, k) in enumerate(raw):
            q = i % nq
            self.pieces.append((st, p0, k, q, qcols[q]))
            qcols[q] += k * 8
        self.NI = max(qcols) if qcols else 8

    def layout(self, npad):
        """Blob column layout (int16 columns)."""
        c = {}
        off = 0
        for name, width in (("x", npad), ("idx", self.NI),
                            ("dstl", self.NCH), ("ew", self.NCH),
                            ("iota", 128), ("w1", D + 1), ("w2", D + 1),
                            ("rows", D + 1)):
            c[name] = off
            off += width
        c["C"] = off
        return c


def _make_schedule_and_arrays(rows_idx, dstl, ew, blk, nblk, half_thresh,
                              ncores, core_of_edge):
    """rows_idx: gather-table row index per edge (global). Returns
    (Schedule, per-core dict of flat padded arrays)."""
    half = (rows_idx >= half_thresh).astype(np.int64)
    key = (core_of_edge * nblk + blk) * 2 + half
    counts = np.bincount(key, minlength=ncores * nblk * 2).reshape(
        ncores, nblk, 2)
    mx = counts.max(axis=0)  # [nblk, 2]
    nlo = _cdiv(mx[:, 0], 128)
    nhi = _cdiv(mx[:, 1], 128)
    force = (nlo + nhi) == 0
    nlo[force] = 1
    sched = Schedule(nlo, nhi)

    lo_base = np.concatenate([[0], np.cumsum(nlo)])[:-1] * 128
    hi_base = np.concatenate([[0], np.cumsum(nhi)])[:-1] * 128

    per_core = []
    for c in range(ncores):
        m = core_of_edge == c
        r, dl, w, b, h = rows_idx[m], dstl[m], ew[m], blk[m], half[m]
        r = np.where(h == 1, r - half_thresh, r)
        order = np.lexsort((r, h, b))
        r, dl, w, b, h = r[order], dl[order], w[order], b[order], h[order]
        gkey = b * 2 + h
        gcnt = np.bincount(gkey, minlength=nblk * 2)
        gstart = np.concatenate([[0], np.cumsum(gcnt)])[:-1]
        pos_in_group = np.arange(len(r)) - gstart[gkey]
        base = np.where(h == 0, lo_base[b], hi_base[b])
        pos = base + pos_in_group

        idx_lo = np.zeros(sched.NLO * 128, np.int16)
        dstl_lo = np.zeros(sched.NLO * 128, np.float32)
        ew_lo = np.zeros(sched.NLO * 128, np.float32)
        idx_hi = np.zeros(max(sched.NHI, 1) * 128, np.int16)
        dstl_hi = np.zeros(max(sched.NHI, 1) * 128, np.float32)
        ew_hi = np.zeros(max(sched.NHI, 1) * 128, np.float32)
        lo = h == 0
        idx_lo[pos[lo]] = r[lo]
        dstl_lo[pos[lo]] = dl[lo]
        ew_lo[pos[lo]] = w[lo]
        hi = ~lo
        idx_hi[pos[hi]] = r[hi]
        dstl_hi[pos[hi]] = dl[hi]
        ew_hi[pos[hi]] = w[hi]
        per_core.append(dict(idx_lo=idx_lo, dstl_lo=dstl_lo, ew_lo=ew_lo,
                             idx_hi=idx_hi, dstl_hi=dstl_hi, ew_hi=ew_hi))
    return sched, per_core


def _bf(a):
    return np.ascontiguousarray(np.asarray(a, np.float32).astype(
        ml_dtypes.bfloat16)).view(np.int16)


def _pack_cols_bf(v):
    """(nch*128,) f32 -> [128, nch] bf16(int16 view): column j = chunk j."""
    return _bf(np.asarray(v).reshape(-1, 128).T)


def build_blob(sched, nblk, x_shard, idx_lo, idx_hi, dstl_lo, dstl_hi,
               ew_lo, ew_hi, W1, b1, W2, b2, g1, be1, g2, be2):
    npad = nblk * 128
    L = sched.layout(npad)
    blob = np.zeros((128, L["C"]), np.int16)

    # x section: [128, npad]; col j*128+f = x_shard[j*128 + p, f]
    xs = np.zeros((128, nblk, D), np.float32)
    n = x_shard.shape[0]
    xr = np.zeros((npad, D), np.float32)
    xr[:n] = x_shard
    xs = xr.reshape(nblk, 128, D).transpose(1, 0, 2)  # [p, j, f]
    blob[:, L["x"]:L["x"] + npad] = _bf(xs.reshape(128, npad))

    # idx section: per piece, 2 copies at the assigned queue's partitions
    streams = {"lo": idx_lo, "hi": idx_hi}
    for st, p0, k, q, qcol in sched.pieces:
        idx = streams[st][p0 * 128:(p0 + k) * 128]
        wrap = np.ascontiguousarray(idx.reshape(k * 8, 16).T)  # [16, k*8]
        c0 = L["idx"] + qcol
        blob[32 * q:32 * q + 16, c0:c0 + k * 8] = wrap
        blob[32 * q + 16:32 * q + 32, c0:c0 + k * 8] = wrap

    # dstl / ew: [128, NCH] bf16, lo chunks then hi chunks
    dstl = np.concatenate([dstl_lo, dstl_hi]) if sched.NHI else dstl_lo
    ew = np.concatenate([ew_lo, ew_hi]) if sched.NHI else ew_lo
    blob[:, L["dstl"]:L["dstl"] + sched.NCH] = _pack_cols_bf(dstl)[:, :sched.NCH]
    blob[:, L["ew"]:L["ew"] + sched.NCH] = _pack_cols_bf(ew)[:, :sched.NCH]

    # iota
    blob[:, L["iota"]:L["iota"] + 128] = _bf(
        np.tile(np.arange(128, dtype=np.float32), (128, 1)))

    # augmented weights [128, 129]
    def aug_w(W):
        W = np.asarray(W, np.float32)
        return np.concatenate([W, W.mean(axis=1, keepdims=True)], 1)

    blob[:, L["w1"]:L["w1"] + D + 1] = _bf(aug_w(W1))
    blob[:, L["w2"]:L["w2"] + D + 1] = _bf(aug_w(W2))

    # rows section: 6 row-vectors at partitions 0..5, 129 cols
    def aug_b(b):
        b = np.asarray(b, np.float32)
        return np.concatenate([b, [b.mean()]])

    rows = np.zeros((6, D + 1), np.float32)
    rows[0] = aug_b(b1)
    rows[1] = aug_b(b2)
    rows[2, :D] = np.asarray(g1, np.float32)
    rows[3, :D] = np.asarray(g2, np.float32)
    rows[4, :D] = np.asarray(be1, np.float32)
    rows[5, :D] = np.asarray(be2, np.float32)
    blob[0:6, L["rows"]:L["rows"] + D + 1] = _bf(rows)
    return blob


def build_kernel(n_nodes, nshard, ncores, sched, piece=PIECE,
                 half_thresh=HALF, dma_scratch=16384, n_queues=NQ,
                 sim1=False, ring_bufs=3, mask_bufs=8, work_bufs=4,
                 bt_bufs=4, agg_bufs=2):
    nblk = _cdiv(nshard, 128)
    npad = nblk * 128
    L = sched.layout(npad)
    nc = bacc.Bacc("TRN2", target_bir_lowering=False, debug=False,
                   num_devices=1 if sim1 else ncores,
                   dynamic_dma_scratch_size=dma_scratch,
                   num_swdge_queues=n_queues)

    blob = nc.dram_tensor("blob", [128, L["C"]], I16, kind="ExternalInput")
    out = nc.dram_tensor("out", [nshard, D], BF16, kind="ExternalOutput")
    chain_in = nc.dram_tensor("chain_in", [1, 128], F32, kind="ExternalInput")
    chain_out = nc.dram_tensor("chain_out", [1, 128], F32,
                               kind="ExternalOutput")

    cc_in = [nc.dram_tensor(f"cc_in{l}", [128, npad], BF16, kind="Internal")
             for l in (0, 1)]
    nrows_cc = 128 * ncores * npad // D
    cc = [nc.dram_tensor(f"cc{l}", [nrows_cc, D], BF16, kind="Internal",
                         addr_space="Shared") for l in (0, 1)]

    def allgather(l):
        out_view = cc[l].ap().rearrange("(q j) f -> q (j f)", j=nblk)
        if sim1:
            nc.sync.dma_start(out_view[0:128, :], cc_in[l].ap())
        else:
            nc.gpsimd.collective_compute(
                "AllGather", mybir.AluOpType.bypass,
                replica_groups=[list(range(ncores))],
                ins=[cc_in[l].ap()], outs=[out_view])

    with tile.TileContext(nc) as tc:
        with (
            tc.tile_pool(name="const", bufs=1) as const,
            tc.tile_pool(name="ring", bufs=ring_bufs) as ring,
            tc.tile_pool(name="mask", bufs=mask_bufs) as maskp,
            tc.tile_pool(name="work", bufs=work_bufs) as work,
            tc.tile_pool(name="stat", bufs=16) as stat,
            tc.tile_pool(name="btps", bufs=bt_bufs, space="PSUM") as btps,
            tc.tile_pool(name="aggps", bufs=agg_bufs, space="PSUM") as aggps,
        ):
            bl = blob.ap()

            def bsec(name, width, dt=BF16):
                return bl[:, L[name]:L[name] + width].bitcast(dt)

            # ship x shard to the AllGather bounce (DRAM->DRAM), collective
            nc.sync.dma_start(cc_in[0].ap(), bsec("x", npad))
            allgather(0)

            idx_s = const.tile([128, sched.NI], I16, tag="idx_s")
            nc.sync.dma_start(idx_s[:], bl[:, L["idx"]:L["idx"] + sched.NI])
            dstl_b = const.tile([128, sched.NCH], BF16, tag="dstl_b")
            nc.sync.dma_start(dstl_b[:], bsec("dstl", sched.NCH))
            ew_b = const.tile([128, sched.NCH], BF16, tag="ew_b")
            nc.sync.dma_start(ew_b[:], bsec("ew", sched.NCH))
            dstl_s = const.tile([128, sched.NCH], F32, tag="dstl_s")
            nc.vector.tensor_copy(dstl_s[:], dstl_b[:])
            ew_s = const.tile([128, sched.NCH], F32, tag="ew_s")
            nc.vector.tensor_copy(ew_s[:], ew_b[:])
            iota_t = const.tile([128, 128], BF16, tag="iota_t")
            nc.sync.dma_start(iota_t[:], bsec("iota", 128))
            w_t = {}
            for l in (1, 2):
                w_t[l] = const.tile([128, D + 1], BF16, tag=f"w{l}_t",
                                    name=f"w{l}_t")
                nc.sync.dma_start(w_t[l][:], bsec(f"w{l}", D + 1))
            rows_t = const.tile([6, D + 1], BF16, tag="rows_t")
            nc.sync.dma_start(rows_t[:],
                              bl[0:6, L["rows"]:L["rows"] + D + 1].bitcast(BF16))

            zero_t = const.tile([128, 1], F32, tag="zero_t")
            nc.vector.memset(zero_t[:], 0.0)
            eps_t = const.tile([128, 1], F32, tag="eps_t")
            nc.vector.memset(eps_t[:], LN_EPS)
            ones1_t = const.tile([1, 128], BF16, tag="ones1_t")
            nc.vector.memset(ones1_t[:], 1.0)

            # move each row-vector to partition 0 (matmul needs base 0/32/64),
            # then broadcast g/be to [128, 128] f32 via K=1 matmuls
            row0 = {}
            for row in range(6):
                t = const.tile([1, D + 1], BF16, tag=f"row0_{row}",
                               name=f"row0_{row}")
                nc.sync.dma_start(t[:], rows_t[row:row + 1, :])
                row0[row] = t
            b_t = {1: row0[0], 2: row0[1]}
            g_t, be_t = {}, {}
            for l, (grow, berow) in {1: (2, 4), 2: (3, 5)}.items():
                for name, row, store in (("g", grow, g_t), ("be", berow, be_t)):
                    ps = btps.tile([128, 128], F32, tag="bt128")
                    nc.tensor.matmul(ps[:], ones1_t[:],
                                     row0[row][:, 0:128],
                                     start=True, stop=True)
                    t = const.tile([128, 128], F32, tag=f"{name}{l}_t",
                                   name=f"{name}{l}_t")
                    nc.scalar.copy(t[:], ps[:])
                    store[l] = t

            h2_all = const.tile([128, nblk, 128], BF16, tag="h2all")
            out_all = const.tile([128, nblk, 128], BF16, tag="outall")

            def emit_gathers(src_lo, src_hi):
                tiles = {}
                for st, p0, k, q, qcol in sched.pieces:
                    t = ring.tile([128, piece, 128], BF16, tag=f"ring{st}")
                    src = src_lo if st == "lo" else src_hi
                    nc.gpsimd.dma_gather(
                        t[:, :k, :], src,
                        idx_s[:, qcol:qcol + k * 8],
                        k * 128, k * 128, D,
                        single_packet=False, queue_num=q)
                    tiles[(st, p0 // piece)] = t
                return tiles

            def do_layer(l, cc_view, write_out):
                pieces = emit_gathers(
                    cc_view[0:min(half_thresh, nrows_cc), :],
                    cc_view[half_thresh:nrows_cc, :]
                    if nrows_cc > half_thresh else cc_view[0:nrows_cc, :])
                lo_pos = hi_pos = 0
                for b in range(nblk):
                    jobs = [("lo", lo_pos + i, 0) for i in range(sched.nlo[b])] + \
                           [("hi", hi_pos + i, sched.NLO)
                            for i in range(sched.nhi[b])]
                    lo_pos += sched.nlo[b]
                    hi_pos += sched.nhi[b]
                    bt = btps.tile([128, 128], F32, tag="bt128")
                    for ji, (st, pos, cbase) in enumerate(jobs):
                        S = maskp.tile([128, 128], BF16, tag="S")
                        c = cbase + pos
                        nc.vector.tensor_scalar(
                            S[:], iota_t[:],
                            dstl_s[:, c:c + 1],
                            ew_s[:, c:c + 1],
                            mybir.AluOpType.is_equal, mybir.AluOpType.mult)
                        src_tile = pieces[(st, pos // piece)]
                        nc.tensor.matmul(
                            bt[:], src_tile[:, pos % piece, :], S[:],
                            start=(ji == 0), stop=(ji == len(jobs) - 1))
                    btt = work.tile([128, 128], BF16, tag="btt")
                    nc.scalar.copy(btt[:], bt[:])
                    agg = aggps.tile([128, D + 1], F32, tag="agg")
                    nc.tensor.matmul(agg[:], btt[:], w_t[l][:],
                                     start=True, stop=False)
                    nc.tensor.matmul(agg[:], ones1_t[:], b_t[l][:],
                                     start=False, stop=True)
                    # LayerNorm (mean comes free in column 128) + relu
                    mu = stat.tile([128, 1], F32, tag="mu")
                    nc.scalar.copy(mu[:], agg[:, D:D + 1])
                    t_ = work.tile([128, 128], F32, tag="t")
                    nc.vector.tensor_scalar(t_[:], agg[:, 0:D], mu[:], None,
                                            mybir.AluOpType.subtract)
                    sq = work.tile([128, 128], F32, tag="sq")
                    var = stat.tile([128, 1], F32, tag="var")
                    nc.scalar.activation(sq[:], t_[:],
                                         mybir.ActivationFunctionType.Square,
                                         bias=zero_t[:], accum_out=var[:])
                    sd = stat.tile([128, 1], F32, tag="sd")
                    nc.scalar.activation(sd[:], var[:],
                                         mybir.ActivationFunctionType.Sqrt,
                                         bias=eps_t[:], scale=1.0 / D)
                    rstd = stat.tile([128, 1], F32, tag="rstd")
                    nc.vector.reciprocal(rstd[:], sd[:])
                    hn = work.tile([128, 128], F32, tag="hn")
                    nc.vector.tensor_scalar(hn[:], t_[:], rstd[:], None,
                                            mybir.AluOpType.mult)
                    hg = work.tile([128, 128], F32, tag="hg")
                    nc.vector.tensor_tensor(hg[:], hn[:], g_t[l][:],
                                            mybir.AluOpType.mult)
                    hb = work.tile([128, 128], F32, tag="hb")
                    nc.vector.tensor_tensor(hb[:], hg[:], be_t[l][:],
                                            mybir.AluOpType.add)
                    write_out(b, hb)

            def l1_write(b, hb):
                nc.scalar.activation(h2_all[:, b, :], hb[:],
                                     mybir.ActivationFunctionType.Relu,
                                     bias=zero_t[:])
                if b == nblk - 1:
                    nc.sync.dma_start(cc_in[1].ap(), h2_all[:])
                    allgather(1)

            def l2_write(b, hb):
                nc.scalar.activation(out_all[:, b, :], hb[:],
                                     mybir.ActivationFunctionType.Relu,
                                     bias=zero_t[:])

            do_layer(1, cc[0].ap(), l1_write)
            do_layer(2, cc[1].ap(), l2_write)

            # write the output shard (node-major [nshard, D])
            nfull = (nshard // 128) * 128
            if nfull:
                nc.sync.dma_start(
                    out.ap()[0:nfull, :].rearrange("(j p) f -> p j f", p=128),
                    out_all[:, 0:nshard // 128, :])
            rem = nshard - nfull
            if rem:
                nc.sync.dma_start(out.ap()[nfull:nshard, :],
                                  out_all[0:rem, nshard // 128, :])
            cht = const.tile([1, 128], F32, tag="cht")
            nc.sync.dma_start(cht[:], chain_in.ap())
            nc.sync.dma_start(chain_out.ap(), cht[:])
    nc.compile()
    return nc


def _phi(n, nshard, nblk):
    """gather-table row in cc for global node n."""
    r = n // nshard
    ld = n % nshard
    j = ld // 128
    p = ld % 128
    return (r * 128 + p) * nblk + j


def prepare(x, edge_index, edge_weight, W1, b1, W2, b2, g1, be1, g2, be2,
            n_nodes=N_NODES, ncores=P, half_thresh=HALF):
    """Host-side prep: returns (sched, in_maps)."""
    nshard = n_nodes // ncores
    nblk = _cdiv(nshard, 128)

    src = np.asarray(edge_index[0]).astype(np.int64)
    dst = np.asarray(edge_index[1]).astype(np.int64)
    ew = np.asarray(edge_weight, np.float32)
    core_of_edge = dst // nshard
    ld = dst % nshard
    blk = ld // 128
    dstl = (ld % 128).astype(np.float32)

    phi = _phi(src, nshard, nblk)
    sched, pc = _make_schedule_and_arrays(
        phi, dstl, ew, blk, nblk, half_thresh, ncores, core_of_edge)

    x = np.asarray(x, np.float32)
    in_maps = []
    for c in range(ncores):
        d = pc[c]
        blob = build_blob(sched, nblk, x[c * nshard:(c + 1) * nshard],
                          d["idx_lo"], d["idx_hi"], d["dstl_lo"],
                          d["dstl_hi"], d["ew_lo"], d["ew_hi"],
                          W1, b1, W2, b2, g1, be1, g2, be2)
        in_maps.append({"blob": blob,
                        "chain_in": np.zeros((1, 128), np.float32)})
    return sched, in_maps


_CACHE = {}


def kernel(x, edge_index, edge_weight, W1, b1, W2, b2, g1, be1, g2, be2):
    n_nodes = int(np.asarray(x).shape[0])
    nshard = n_nodes // P
    sched, in_maps = prepare(
        x, edge_index, edge_weight, W1, b1, W2, b2, g1, be1, g2, be2,
        n_nodes=n_nodes)
    key = (n_nodes, tuple(sched.nlo), tuple(sched.nhi))
    if key not in _CACHE:
        _CACHE[key] = build_kernel(n_nodes, nshard, P, sched)
    nc = _CACHE[key]
    res = run_bass_kernel_spmd(nc, in_maps, core_ids=list(range(P)))
    return np.concatenate(
        [np.asarray(res.results[c]["out"]).astype(np.float32)
         for c in range(P)], axis=0)


if __name__ == "__main__":
    import reference
    inputs = {k: np.asarray(v) for k, v in reference.setup_inputs().items()}
    out = kernel(**inputs)
    exp = np.asarray(reference.reference(**inputs))
    err = np.abs(out - exp).max()
    denom = np.abs(exp).max()
    print("max abs err:", err, "rel:", err / denom)
